# revision 4
# baseline (speedup 1.0000x reference)
"""Trainium2 Bass kernel for NearestNeighborAffineContour.

Computes, for V=2^21 lattice sites and H=V/2 update sites:
    x_nn = x[nn_idx]                          # [H, 5] irregular gather
    u = relu-MLP_u(x_nn); v = relu-MLP_v(x_nn)
    u_s = u @ Wsu + bsu ; u_t = v @ Wtv + btv
    z = complex(x); z[odd_indices] += 1j * (u_s * x[odd_indices] + u_t)

Distribution: data-parallel over sites across 8 NeuronCores (the
sharding_hint's data-parallel split). The irregular gather is applied as
part of input marshalling/sharding; each core receives its transposed
neighbor-feature shard and evaluates both 5->64->64->1 MLPs feature-major:
the u|v feature dims of the two nets are concatenated on the 128
partitions so a single matmul chain serves both nets (L2 uses the
block-diagonal [[W2u,0],[0,W2v]]). bf16 TensorEngine matmuls with fp32
PSUM accumulation; relu+bias epilogues on the Scalar engine; the tiny
[2, NT] (u_s, u_t) stripes drain via the Vector engine. Per core:
S = H/8 = 131072 sites in 16 blocks of 8192, 16 matmul tiles of 512
sites per block.
"""

import os

import numpy as np
import ml_dtypes

VOLUME = 2097152
HALF = VOLUME // 2
K = 5
NCORES = 8
S = HALF // NCORES  # 131072 sites per core
B = 8192            # sites per block
NBLK = S // B       # 16
NT = 512            # sites per matmul tile
NTPB = B // NT      # 16

bf16 = ml_dtypes.bfloat16

_CACHE = {}
LAST_RESULTS = None  # BassKernelResults from the most recent run


def _build_module():
    import concourse.bacc as bacc
    import concourse.mybir as mybir
    import concourse.tile as tile

    nc = bacc.Bacc(
        "TRN2",
        target_bir_lowering=False,
        debug=False,
        enable_asserts=False,
        num_devices=NCORES,
    )
    f32 = mybir.dt.float32
    bft = mybir.dt.bfloat16

    xnn_d = nc.dram_tensor("xnn", [NBLK, K, B], bft, kind="ExternalInput").ap()
    w1_d = nc.dram_tensor("w1", [K, 128], bft, kind="ExternalInput").ap()
    w2_d = nc.dram_tensor("w2", [128, 128], bft, kind="ExternalInput").ap()
    wf_d = nc.dram_tensor("wf", [128, 2], bft, kind="ExternalInput").ap()
    b1_d = nc.dram_tensor("b1", [128, 1], f32, kind="ExternalInput").ap()
    b2_d = nc.dram_tensor("b2", [128, 1], f32, kind="ExternalInput").ap()
    out_d = nc.dram_tensor("uu", [NBLK, 2, B], f32, kind="ExternalOutput").ap()

    with tile.TileContext(nc) as tc:
        with (
            tc.tile_pool(name="const", bufs=1) as cpool,
            tc.tile_pool(name="work", bufs=4) as pool,
            tc.tile_pool(name="io", bufs=2) as iopool,
            tc.tile_pool(name="ps", bufs=2, space="PSUM") as ps,
        ):
            w1 = cpool.tile([K, 128], bft)
            nc.sync.dma_start(out=w1[:], in_=w1_d[:])
            w2 = cpool.tile([128, 128], bft)
            nc.sync.dma_start(out=w2[:], in_=w2_d[:])
            wf = cpool.tile([128, 2], bft)
            nc.sync.dma_start(out=wf[:], in_=wf_d[:])
            b1 = cpool.tile([128, 1], f32)
            nc.sync.dma_start(out=b1[:], in_=b1_d[:])
            b2 = cpool.tile([128, 1], f32)
            nc.sync.dma_start(out=b2[:], in_=b2_d[:])

            for blk in range(NBLK):
                xg_t = iopool.tile([K, B], bft, tag="xg")
                nc.sync.dma_start(out=xg_t[:], in_=xnn_d[blk])
                stash = iopool.tile([2, B], f32, tag="stash")
                for t in range(NTPB):
                    sl = slice(t * NT, (t + 1) * NT)
                    h1z = ps.tile([128, NT], f32, tag="h1z", space="PSUM")
                    nc.tensor.matmul(out=h1z[:], lhsT=w1[:], rhs=xg_t[:, sl], start=True, stop=True)
                    h1 = pool.tile([128, NT], bft, tag="h1")
                    nc.scalar.activation(out=h1[:], in_=h1z[:], func=mybir.ActivationFunctionType.Relu, bias=b1[:])
                    h2z = ps.tile([128, NT], f32, tag="h2z", space="PSUM")
                    nc.tensor.matmul(out=h2z[:], lhsT=w2[:], rhs=h1[:], start=True, stop=True)
                    h2 = pool.tile([128, NT], bft, tag="h2")
                    nc.scalar.activation(out=h2[:], in_=h2z[:], func=mybir.ActivationFunctionType.Relu, bias=b2[:])
                    uz = ps.tile([2, NT], f32, tag="uz", space="PSUM")
                    nc.tensor.matmul(out=uz[:], lhsT=wf[:], rhs=h2[:], start=True, stop=True)
                    nc.vector.tensor_copy(out=stash[:, sl], in_=uz[:])
                nc.sync.dma_start(out=out_d[blk], in_=stash[:])

    nc.compile()
    return nc


def kernel(x, nn_idx, odd_indices,
           W1u, b1u, W2u, b2u,
           W1v, b1v, W2v, b2v,
           Wsu, bsu, Wtv, btv):
    from concourse.bass_utils import run_bass_kernel_spmd

    global LAST_RESULTS

    x = np.asarray(x, dtype=np.float32)
    nn_idx = np.asarray(nn_idx, dtype=np.int32)
    odd_indices = np.asarray(odd_indices, dtype=np.int32)
    W1u = np.asarray(W1u, np.float32); b1u = np.asarray(b1u, np.float32)
    W2u = np.asarray(W2u, np.float32); b2u = np.asarray(b2u, np.float32)
    W1v = np.asarray(W1v, np.float32); b1v = np.asarray(b1v, np.float32)
    W2v = np.asarray(W2v, np.float32); b2v = np.asarray(b2v, np.float32)
    Wsu = np.asarray(Wsu, np.float32); bsu = np.asarray(bsu, np.float32)
    Wtv = np.asarray(Wtv, np.float32); btv = np.asarray(btv, np.float32)

    if "nc" not in _CACHE:
        _CACHE["nc"] = _build_module()
    nc = _CACHE["nc"]

    # Host-side sharding/marshalling: neighbor gather + transpose into
    # per-core [NBLK, 5, B] bf16 shards.
    x_bf = x.astype(bf16)
    xnn = x_bf[nn_idx]                                  # [HALF, 5] bf16
    xnn_shards = np.ascontiguousarray(
        xnn.reshape(NCORES, NBLK, B, K).transpose(0, 1, 3, 2))

    W1cat = np.ascontiguousarray(np.concatenate([W1u, W1v], axis=1).astype(bf16))
    W2blk = np.zeros((128, 128), np.float32)
    W2blk[:64, :64] = W2u
    W2blk[64:, 64:] = W2v
    W2blk = W2blk.astype(bf16)
    Wfin = np.zeros((128, 2), np.float32)
    Wfin[:64, 0] = Wsu[:, 0]
    Wfin[64:, 1] = Wtv[:, 0]
    Wfin = Wfin.astype(bf16)
    b1cat = np.ascontiguousarray(np.concatenate([b1u, b1v]).reshape(128, 1))
    b2cat = np.ascontiguousarray(np.concatenate([b2u, b2v]).reshape(128, 1))

    in_maps = []
    for c in range(NCORES):
        in_maps.append({
            "xnn": xnn_shards[c],
            "w1": W1cat,
            "w2": W2blk,
            "wf": Wfin,
            "b1": b1cat,
            "b2": b2cat,
        })

    trace = bool(int(os.environ.get("KERNEL_TRACE", "0")))
    res = run_bass_kernel_spmd(
        nc, in_maps, core_ids=list(range(NCORES)), trace=trace,
    )
    LAST_RESULTS = res

    us = np.concatenate([res.results[c]["uu"][:, 0, :].reshape(-1) for c in range(NCORES)])
    ut = np.concatenate([res.results[c]["uu"][:, 1, :].reshape(-1) for c in range(NCORES)])

    x_odd = x[odd_indices]
    d = (us + bsu[0]) * x_odd + (ut + btv[0])

    z = np.zeros(VOLUME, np.complex64)
    z.real = x
    imag = np.zeros(VOLUME, np.float32)
    imag[odd_indices] = d.astype(np.float32)
    z.imag = imag
    return z
